# revision 5
# baseline (speedup 1.0000x reference)
"""Cross multi-head attention Trainium2 kernel (8-core SPMD).

Problem (hardcoded): B=4, SQ=1024, SKV=4096, DIM_Q=1024, DIM_KV=768, H=16,
DH=64, DIM_OUT=256.  Returns (att_output [B,SQ,256] f32,
att_weights [B,H,SQ,SKV] f32) matching the torch-style reference:

    q = Q @ Wq.T + bq ; k = K @ Wk.T + bk ; v = V @ Wv.T + bv
    scores = q @ k.T / sqrt(DH); masked (att_mask True -> -1e9); softmax
    att = softmax @ v ; out = att @ Wo.T + bo

Sharding: core c handles batch b=c//2 and head half hh=c%2 (8 heads,
hidden slice of 512).  Wq/Wk/Wv column-sharded by head, Wo row-sharded;
the pair all-reduce for Wo is done on the host during unshard (partials
are summed), with bo split evenly between the pair.

Device-side layout is fully transposed ("layout B"): scoresT [kv, q] per
head, so softmax+attV need no on-chip transposes.  The 1 GiB att_weights
output is written as [h, kv, q] fp16 and transposed/cast on the host.

Algebraic simplifications (exact w.r.t. the reference):
  - bk shifts every score row by a per-q constant -> softmax invariant -> drop.
  - bv contributes bv @ Wo.T to the output -> folded into bo on the host.
  - softmax needs no max-subtraction: scores are O(5), exp cannot overflow.
"""

import os
import sys
import numpy as np

sys.path.insert(0, "/opt/trn_rl_repo")

import concourse.bacc as bacc
import concourse.bass as bass
import concourse.mybir as mybir
import concourse.tile as tile
from concourse.bass_utils import run_bass_kernel_spmd

B, SQ, SKV = 4, 1024, 4096
DIM_Q, DIM_KV, H, DIM_OUT = 1024, 768, 16, 256
DH = DIM_Q // H            # 64
HL = 8                     # heads per core (local)
S = HL * DH                # 512 hidden slice per core
N_CORES = 8

FP16 = mybir.dt.float16
F32 = mybir.dt.float32

KQT = DIM_Q // 128     # 8  k-tiles for q projection
KKV = DIM_KV // 128    # 6  k-tiles for k/v projections
JH = S // 128          # 4  hidden 128-tiles
NKV = SKV // 128       # 32 kv 128-tiles
NP1 = 512              # free-dim tile for phase-1 projections
NQ2 = 256              # q free-dim tile for phase-2 (keeps pm hold small)
VROW = HL * (DH + 1)   # 520: per-head dh cols + ones col

_CACHED_NC = None


def _build_program():
    """Build + compile the per-core Bass program (same for all 8 cores)."""
    nc = bacc.Bacc("TRN2", target_bir_lowering=False, debug=False,
                   num_devices=N_CORES)

    # ---- DRAM I/O ----------------------------------------------------
    qt_d = nc.dram_tensor("qt", [DIM_Q, SQ], FP16, kind="ExternalInput")
    kt_d = nc.dram_tensor("kt", [DIM_KV, SKV], FP16, kind="ExternalInput")
    vt_d = nc.dram_tensor("vt", [DIM_KV, SKV], FP16, kind="ExternalInput")
    maskt_d = nc.dram_tensor("maskt", [SKV, SQ], FP16, kind="ExternalInput")
    wqt_d = nc.dram_tensor("wqt", [DIM_Q, S], FP16, kind="ExternalInput")
    wkt_d = nc.dram_tensor("wkt", [DIM_KV, S], FP16, kind="ExternalInput")
    wvt_d = nc.dram_tensor("wvt", [DIM_KV, S], FP16, kind="ExternalInput")
    wot_d = nc.dram_tensor("wot", [S, DIM_OUT], FP16, kind="ExternalInput")
    bq_d = nc.dram_tensor("bq", [128, S // 128], F32, kind="ExternalInput")
    bo_d = nc.dram_tensor("bo", [1, DIM_OUT], F32, kind="ExternalInput")

    awt_d = nc.dram_tensor("awt", [HL, SKV, SQ], FP16, kind="ExternalOutput")
    outp_d = nc.dram_tensor("outp", [SQ, DIM_OUT], F32, kind="ExternalOutput")

    with tile.TileContext(nc) as tc:
        with (
            tc.tile_pool(name="persist", bufs=1) as persist,
        ):
            # ---- persistent SBUF tensors -----------------------------
            qt_sb = persist.tile([128, JH, SQ], FP16, tag="qt_sb")
            kt_sb = persist.tile([128, JH, SKV], FP16, tag="kt_sb")
            v_sb = persist.tile([128, NKV, VROW], FP16, tag="v_sb")
            bq_sb = persist.tile([128, S // 128], F32, tag="bq_sb")
            bo_sb = persist.tile([128, DIM_OUT], F32, tag="bo_sb")
            wot_sb = persist.tile([128, JH, DIM_OUT], FP16, tag="wot_sb")
            attn_sb = persist.tile([128, JH, SQ], FP16, tag="attn_sb")

            nc.sync.dma_start(out=bq_sb[:], in_=bq_d[:])
            nc.gpsimd.dma_start(out=bo_sb[:],
                                in_=bo_d.ap()[0:1, :].partition_broadcast(128))
            nc.sync.dma_start(
                out=wot_sb[:],
                in_=wot_d.ap().rearrange("(j p) n -> p j n", p=128))

            # ones columns of v_sb (col DH of each head block)
            nc.vector.memset(
                v_sb[:].rearrange("p t (h e) -> p t h e", e=DH + 1)
                [:, :, :, DH:DH + 1], 1.0)

            # ============ phase 1: projections (scoped pools) =========
            with (
                tc.tile_pool(name="wstage", bufs=1) as wstage,
                tc.tile_pool(name="stream", bufs=3) as stream,
                tc.tile_pool(name="psum", bufs=4, space="PSUM") as psum,
            ):
                # -- qT = (Wq_s @ Q^T) + bq -> [S, SQ] --
                wqt_sb = wstage.tile([128, KQT, S], FP16, tag="wqt")
                nc.sync.dma_start(
                    out=wqt_sb[:],
                    in_=wqt_d.ap().rearrange("(k p) m -> p k m", p=128))
                qtin_sb = wstage.tile([128, KQT, SQ], FP16, tag="qtin")
                nc.sync.dma_start(
                    out=qtin_sb[:],
                    in_=qt_d.ap().rearrange("(k p) n -> p k n", p=128))
                for j in range(JH):
                    for n in range(SQ // NP1):
                        ps = psum.tile([128, NP1], F32, tag="ps_proj")
                        for k in range(KQT):
                            nc.tensor.matmul(
                                ps[:],
                                wqt_sb[:, k, j * 128:(j + 1) * 128],
                                qtin_sb[:, k, n * NP1:(n + 1) * NP1],
                                start=(k == 0), stop=(k == KQT - 1))
                        nc.scalar.activation(
                            qt_sb[:, j, n * NP1:(n + 1) * NP1], ps[:],
                            mybir.ActivationFunctionType.Identity,
                            bias=bq_sb[:, j:j + 1])

                # -- kT = Wk_s @ K^T -> [S, SKV] (bk dropped) --
                wkt_sb = wstage.tile([128, KKV, S], FP16, tag="wkt")
                nc.sync.dma_start(
                    out=wkt_sb[:],
                    in_=wkt_d.ap().rearrange("(k p) m -> p k m", p=128))
                for n in range(SKV // NP1):
                    ktin = stream.tile([128, KKV, NP1], FP16, tag="ktin")
                    nc.sync.dma_start(
                        out=ktin[:],
                        in_=kt_d.ap().rearrange("(k p) n -> p k n", p=128)
                        [:, :, n * NP1:(n + 1) * NP1])
                    for j in range(JH):
                        ps = psum.tile([128, NP1], F32, tag="ps_proj")
                        for k in range(KKV):
                            nc.tensor.matmul(
                                ps[:],
                                wkt_sb[:, k, j * 128:(j + 1) * 128],
                                ktin[:, k, :],
                                start=(k == 0), stop=(k == KKV - 1))
                        nc.scalar.copy(kt_sb[:, j, n * NP1:(n + 1) * NP1],
                                       ps[:])

                # -- v = (V @ Wv_s^T) -> [kv, 520] (bv folded into bo) --
                wvt_sb = wstage.tile([128, KKV, S], FP16, tag="wvt")
                nc.sync.dma_start(
                    out=wvt_sb[:],
                    in_=wvt_d.ap().rearrange("(k p) m -> p k m", p=128))
                for kvt in range(NKV):
                    vtin = stream.tile([128, KKV, 128], FP16, tag="vtin")
                    nc.sync.dma_start(
                        out=vtin[:],
                        in_=vt_d.ap().rearrange("(k p) n -> p k n", p=128)
                        [:, :, kvt * 128:(kvt + 1) * 128])
                    ps = psum.tile([128, S], F32, tag="ps_proj")
                    for k in range(KKV):
                        nc.tensor.matmul(
                            ps[:], vtin[:, k, :], wvt_sb[:, k, :],
                            start=(k == 0), stop=(k == KKV - 1))
                    nc.scalar.copy(
                        v_sb[:, kvt, :].rearrange("p (h e) -> p h e",
                                                  e=DH + 1)[:, :, 0:DH],
                        ps[:].rearrange("p (h d) -> p h d", d=DH))

            # ============ phase 2: attention ==========================
            with (
                tc.tile_pool(name="mask_pool", bufs=1) as mask_pool,
                tc.tile_pool(name="pm_pool", bufs=2 * NKV + 2) as pm_pool,
                tc.tile_pool(name="small", bufs=3) as small,
                tc.tile_pool(name="rec", bufs=2) as rec,
                tc.tile_pool(name="outstage", bufs=6) as outstage,
                tc.tile_pool(name="psum_sc", bufs=4, space="PSUM") as psum_sc,
                tc.tile_pool(name="psum_att", bufs=2, space="PSUM") as psatt,
                tc.tile_pool(name="psum_out", bufs=2, space="PSUM") as psum_out,
            ):
                for qt in range(SQ // NQ2):
                    q0 = qt * NQ2
                    mask_sb = mask_pool.tile([128, NKV, NQ2], FP16,
                                             tag="mask_sb")
                    nc.sync.dma_start(
                        out=mask_sb[:],
                        in_=maskt_d.ap().rearrange("(t p) q -> p t q", p=128)
                        [:, :, q0:q0 + NQ2])
                    for hp in range(JH):
                        hA, hB = 2 * hp, 2 * hp + 1
                        att_ps = {}
                        pm_tiles = {}
                        for h in (hA, hB):
                            att_ps[h] = psatt.tile(
                                [DH + 1, NQ2], F32, tag="att_ps",
                                name=f"attps_{qt}_{h}")
                        for kvt in range(NKV):
                            for h, base in ((hA, 0), (hB, 64)):
                                ps = psum_sc.tile([128, NQ2], F32, tag="ps_sc")
                                nc.tensor.matmul(
                                    ps[:],
                                    kt_sb[base:base + DH, hp,
                                          kvt * 128:(kvt + 1) * 128],
                                    qt_sb[base:base + DH, hp, q0:q0 + NQ2],
                                    start=True, stop=True)
                                p_t = small.tile([128, NQ2], FP16,
                                                 tag="p_exp")
                                nc.scalar.activation(
                                    p_t[:], ps[:],
                                    mybir.ActivationFunctionType.Exp,
                                    scale=float(1.0 / np.sqrt(DH)))
                                pm = pm_pool.tile([128, NQ2], FP16, tag="pm",
                                                  name=f"pm_{h}_{kvt}")
                                nc.vector.tensor_mul(pm[:], p_t[:],
                                                     mask_sb[:, kvt, :])
                                pm_tiles[(h, kvt)] = pm
                                nc.tensor.matmul(
                                    att_ps[h][:],
                                    v_sb[:, kvt,
                                         (h % HL) * (DH + 1):
                                         (h % HL) * (DH + 1) + DH + 1],
                                    pm[:],
                                    start=(kvt == 0), stop=(kvt == NKV - 1))
                        # softmax sums live in row DH of att psum
                        for h in (hA, hB):
                            rrow = rec.tile([1, NQ2], F32, tag="rrow")
                            rscr = rec.tile([1, NQ2], F32, tag="rscr")
                            rre = rec.tile([1, NQ2], F32, tag="rrec")
                            nc.vector.tensor_copy(rrow[:],
                                                  att_ps[h][DH:DH + 1, :])
                            nc.vector.reciprocal_approx_accurate(
                                out=rre[:], in_=rrow[:], scratch=rscr[:])
                            rb32 = rec.tile([128, NQ2], F32, tag="rb32")
                            nc.gpsimd.partition_broadcast(rb32[:], rre[:])
                            rb16 = rec.tile([128, NQ2], FP16, tag="rb16")
                            nc.vector.tensor_copy(rb16[:], rb32[:])
                            for kvt in range(NKV):
                                ow = outstage.tile([128, NQ2], FP16, tag="ow")
                                nc.vector.tensor_mul(
                                    ow[:], pm_tiles[(h, kvt)][:], rb16[:])
                                nc.sync.dma_start(
                                    out=awt_d.ap()
                                    [h, kvt * 128:(kvt + 1) * 128,
                                     q0:q0 + NQ2],
                                    in_=ow[:])
                            nc.vector.tensor_mul(
                                attn_sb[(h % 2) * 64:(h % 2) * 64 + DH,
                                        h // 2, q0:q0 + NQ2],
                                att_ps[h][0:DH, :], rb32[0:DH, :])

                    # ---- output projection for this q quarter --------
                    for st in range(NQ2 // 128):
                        sq0 = q0 + st * 128
                        pso = psum_out.tile([128, DIM_OUT], F32, tag="ps_out")
                        for j in range(JH):
                            nc.tensor.matmul(
                                pso[:],
                                attn_sb[:, j, sq0:sq0 + 128],
                                wot_sb[:, j, :],
                                start=(j == 0), stop=(j == JH - 1))
                        oout = outstage.tile([128, DIM_OUT], F32, tag="oout")
                        nc.vector.tensor_add(oout[:], pso[:], bo_sb[:])
                        nc.sync.dma_start(out=outp_d.ap()[sq0:sq0 + 128, :],
                                          in_=oout[:])

    nc.compile()
    return nc


def _get_nc():
    global _CACHED_NC
    if _CACHED_NC is None:
        _CACHED_NC = _build_program()
    return _CACHED_NC


def _prep_core_inputs(c, Q, K, V, att_mask, Wq, bq, Wk, Wv, Wo, bo, bv):
    b, hh = c // 2, c % 2
    sl = slice(hh * S, (hh + 1) * S)
    keep = (~att_mask[b]).astype(np.float16)          # [SKV, SQ] 1=keep
    bo_eff = (bo / 2.0 + Wo[:, sl].astype(np.float64) @
              bv[sl].astype(np.float64)).astype(np.float32)
    return {
        "qt": np.ascontiguousarray(Q[b].T).astype(np.float16),
        "kt": np.ascontiguousarray(K[b].T).astype(np.float16),
        "vt": np.ascontiguousarray(V[b].T).astype(np.float16),
        "maskt": keep,
        "wqt": np.ascontiguousarray(Wq[sl, :].T).astype(np.float16),
        "wkt": np.ascontiguousarray(Wk[sl, :].T).astype(np.float16),
        "wvt": np.ascontiguousarray(Wv[sl, :].T).astype(np.float16),
        "wot": np.ascontiguousarray(Wo[:, sl].T).astype(np.float16),
        "bq": np.ascontiguousarray(
            bq[sl].astype(np.float32).reshape(S // 128, 128).T),
        "bo": bo_eff.reshape(1, DIM_OUT),
    }


def kernel(Q, K, V, att_mask, Wq, bq, Wk, bk, Wv, bv, Wo, bo, _trace=False):
    Q = np.asarray(Q, np.float32)
    K = np.asarray(K, np.float32)
    V = np.asarray(V, np.float32)
    att_mask = np.asarray(att_mask, bool)
    Wq, Wk, Wv, Wo = (np.asarray(a, np.float32) for a in (Wq, Wk, Wv, Wo))
    bq, bk, bv, bo = (np.asarray(a, np.float32) for a in (bq, bk, bv, bo))
    # bk drops out of softmax exactly; bv is folded into bo_eff.

    nc = _get_nc()
    in_maps = [_prep_core_inputs(c, Q, K, V, att_mask, Wq, bq, Wk, Wv, Wo,
                                 bo, bv) for c in range(N_CORES)]
    res = run_bass_kernel_spmd(nc, in_maps, core_ids=list(range(N_CORES)),
                               trace=_trace)

    att_weights = np.empty((B, H, SQ, SKV), np.float32)
    att_output = np.zeros((B, SQ, DIM_OUT), np.float32)
    try:
        import jax.numpy as jnp

        def tr(a):  # [HL, SKV, SQ] fp16 -> [HL, SQ, SKV] f32 (multithreaded)
            return np.asarray(jnp.transpose(jnp.asarray(a), (0, 2, 1))
                              .astype(jnp.float32))
    except Exception:
        def tr(a):
            return a.transpose(0, 2, 1).astype(np.float32)

    for c in range(N_CORES):
        b, hh = c // 2, c % 2
        att_weights[b, hh * HL:(hh + 1) * HL] = tr(res.results[c]["awt"])
        att_output[b] += res.results[c]["outp"]

    if _trace:
        kernel._last_results = res
    return att_output, att_weights


# revision 7
# speedup vs baseline: 1.2953x; 1.2953x over previous
"""Cross multi-head attention Trainium2 kernel (8-core SPMD).

Problem (hardcoded): B=4, SQ=1024, SKV=4096, DIM_Q=1024, DIM_KV=768, H=16,
DH=64, DIM_OUT=256.  Returns (att_output [B,SQ,256] f32,
att_weights [B,H,SQ,SKV] f32) matching the torch-style reference:

    q = Q @ Wq.T + bq ; k = K @ Wk.T + bk ; v = V @ Wv.T + bv
    scores = q @ k.T / sqrt(DH); masked (att_mask True -> -1e9); softmax
    att = softmax @ v ; out = att @ Wo.T + bo

Sharding: core c handles batch b=c//2 and head half hh=c%2 (8 heads,
hidden slice of 512).  Wq/Wk/Wv column-sharded by head, Wo row-sharded;
the pair all-reduce for Wo is done on the host during unshard (partials
are summed), with bo split evenly between the pair.

Device-side layout is fully transposed ("layout B"): scoresT [kv, q] per
head, so softmax+attV need no on-chip transposes.  The 1 GiB att_weights
output is written as [h, kv, q] fp16 and transposed/cast on the host.

Algebraic simplifications (exact w.r.t. the reference):
  - bk shifts every score row by a per-q constant -> softmax invariant -> drop.
  - bv contributes bv @ Wo.T to the output -> folded into bo on the host.
  - softmax needs no max-subtraction: scores are O(5), exp cannot overflow.
"""

import os
import sys
import numpy as np

sys.path.insert(0, "/opt/trn_rl_repo")

import concourse.bacc as bacc
import concourse.bass as bass
import concourse.mybir as mybir
import concourse.tile as tile
from concourse.bass_utils import run_bass_kernel_spmd

B, SQ, SKV = 4, 1024, 4096
DIM_Q, DIM_KV, H, DIM_OUT = 1024, 768, 16, 256
DH = DIM_Q // H            # 64
HL = 8                     # heads per core (local)
S = HL * DH                # 512 hidden slice per core
N_CORES = 8

FP16 = mybir.dt.float16
F32 = mybir.dt.float32

KQT = DIM_Q // 128     # 8  k-tiles for q projection
KKV = DIM_KV // 128    # 6  k-tiles for k/v projections
JH = S // 128          # 4  hidden 128-tiles
NKV = SKV // 128       # 32 kv 128-tiles
NP1 = 512              # free-dim tile for phase-1 projections
NQ2 = 256              # q free-dim tile for phase-2 (keeps pm hold small)
VROW = HL * (DH + 1)   # 520: per-head dh cols + ones col

_CACHED_NC = None


def _build_program():
    """Build + compile the per-core Bass program (same for all 8 cores)."""
    nc = bacc.Bacc("TRN2", target_bir_lowering=False, debug=False,
                   num_devices=N_CORES)

    # ---- DRAM I/O ----------------------------------------------------
    qt_d = nc.dram_tensor("qt", [DIM_Q, SQ], FP16, kind="ExternalInput")
    kt_d = nc.dram_tensor("kt", [DIM_KV, SKV], FP16, kind="ExternalInput")
    vt_d = nc.dram_tensor("vt", [DIM_KV, SKV], FP16, kind="ExternalInput")
    maskt_d = nc.dram_tensor("maskt", [SKV, SQ], FP16, kind="ExternalInput")
    wqt_d = nc.dram_tensor("wqt", [DIM_Q, S], FP16, kind="ExternalInput")
    wkt_d = nc.dram_tensor("wkt", [DIM_KV, S], FP16, kind="ExternalInput")
    wvt_d = nc.dram_tensor("wvt", [DIM_KV, S], FP16, kind="ExternalInput")
    wot_d = nc.dram_tensor("wot", [S, DIM_OUT], FP16, kind="ExternalInput")
    bq_d = nc.dram_tensor("bq", [128, S // 128], F32, kind="ExternalInput")
    bo_d = nc.dram_tensor("bo", [1, DIM_OUT], F32, kind="ExternalInput")

    awt_d = nc.dram_tensor("awt", [HL, SKV, SQ], FP16, kind="ExternalOutput")
    outp_d = nc.dram_tensor("outp", [SQ, DIM_OUT], F32, kind="ExternalOutput")

    with tile.TileContext(nc) as tc:
        with (
            tc.tile_pool(name="persist", bufs=1) as persist,
        ):
            # ---- persistent SBUF tensors -----------------------------
            qt_sb = persist.tile([128, JH, SQ], FP16, tag="qt_sb")
            kt_sb = persist.tile([128, JH, SKV], FP16, tag="kt_sb")
            v_sb = persist.tile([128, NKV, VROW], FP16, tag="v_sb")
            bq_sb = persist.tile([128, S // 128], F32, tag="bq_sb")
            bo_sb = persist.tile([128, DIM_OUT], F32, tag="bo_sb")
            wot_sb = persist.tile([128, JH, DIM_OUT], FP16, tag="wot_sb")
            attn_sb = persist.tile([128, JH, SQ], FP16, tag="attn_sb")

            nc.sync.dma_start(out=bq_sb[:], in_=bq_d[:])
            nc.gpsimd.dma_start(out=bo_sb[:],
                                in_=bo_d.ap()[0:1, :].partition_broadcast(128))
            nc.sync.dma_start(
                out=wot_sb[:],
                in_=wot_d.ap().rearrange("(j p) n -> p j n", p=128))

            # ones columns of v_sb (col DH of each head block)
            nc.vector.memset(
                v_sb[:].rearrange("p t (h e) -> p t h e", e=DH + 1)
                [:, :, :, DH:DH + 1], 1.0)

            # ============ phase 1: projections (scoped pools) =========
            with (
                tc.tile_pool(name="wstage", bufs=1) as wstage,
                tc.tile_pool(name="stream", bufs=3) as stream,
                tc.tile_pool(name="psum", bufs=4, space="PSUM") as psum,
            ):
                # -- qT = (Wq_s @ Q^T) + bq -> [S, SQ] --
                wqt_sb = wstage.tile([128, KQT, S], FP16, tag="wqt")
                nc.sync.dma_start(
                    out=wqt_sb[:],
                    in_=wqt_d.ap().rearrange("(k p) m -> p k m", p=128))
                qtin_sb = wstage.tile([128, KQT, SQ], FP16, tag="qtin")
                nc.sync.dma_start(
                    out=qtin_sb[:],
                    in_=qt_d.ap().rearrange("(k p) n -> p k n", p=128))
                for j in range(JH):
                    for n in range(SQ // NP1):
                        ps = psum.tile([128, NP1], F32, tag="ps_proj")
                        for k in range(KQT):
                            nc.tensor.matmul(
                                ps[:],
                                wqt_sb[:, k, j * 128:(j + 1) * 128],
                                qtin_sb[:, k, n * NP1:(n + 1) * NP1],
                                start=(k == 0), stop=(k == KQT - 1))
                        nc.scalar.activation(
                            qt_sb[:, j, n * NP1:(n + 1) * NP1], ps[:],
                            mybir.ActivationFunctionType.Identity,
                            bias=bq_sb[:, j:j + 1])

                # -- kT = Wk_s @ K^T -> [S, SKV] (bk dropped) --
                wkt_sb = wstage.tile([128, KKV, S], FP16, tag="wkt")
                nc.sync.dma_start(
                    out=wkt_sb[:],
                    in_=wkt_d.ap().rearrange("(k p) m -> p k m", p=128))
                for n in range(SKV // NP1):
                    ktin = stream.tile([128, KKV, NP1], FP16, tag="ktin")
                    nc.sync.dma_start(
                        out=ktin[:],
                        in_=kt_d.ap().rearrange("(k p) n -> p k n", p=128)
                        [:, :, n * NP1:(n + 1) * NP1])
                    for j in range(JH):
                        ps = psum.tile([128, NP1], F32, tag="ps_proj")
                        for k in range(KKV):
                            nc.tensor.matmul(
                                ps[:],
                                wkt_sb[:, k, j * 128:(j + 1) * 128],
                                ktin[:, k, :],
                                start=(k == 0), stop=(k == KKV - 1))
                        nc.scalar.copy(kt_sb[:, j, n * NP1:(n + 1) * NP1],
                                       ps[:])

                # -- v = (V @ Wv_s^T) -> [kv, 520] (bv folded into bo) --
                wvt_sb = wstage.tile([128, KKV, S], FP16, tag="wvt")
                nc.sync.dma_start(
                    out=wvt_sb[:],
                    in_=wvt_d.ap().rearrange("(k p) m -> p k m", p=128))
                for kvt in range(NKV):
                    vtin = stream.tile([128, KKV, 128], FP16, tag="vtin")
                    nc.sync.dma_start(
                        out=vtin[:],
                        in_=vt_d.ap().rearrange("(k p) n -> p k n", p=128)
                        [:, :, kvt * 128:(kvt + 1) * 128])
                    ps = psum.tile([128, S], F32, tag="ps_proj")
                    for k in range(KKV):
                        nc.tensor.matmul(
                            ps[:], vtin[:, k, :], wvt_sb[:, k, :],
                            start=(k == 0), stop=(k == KKV - 1))
                    nc.scalar.copy(
                        v_sb[:, kvt, :].rearrange("p (h e) -> p h e",
                                                  e=DH + 1)[:, :, 0:DH],
                        ps[:].rearrange("p (h d) -> p h d", d=DH))

            # ============ phase 2: attention ==========================
            KB = 4                       # kv-tiles batched per exp/mask op
            NB = NKV // KB               # 8 batches per (qt, hp)
            with (
                tc.tile_pool(name="mask_pool", bufs=1) as mask_pool,
                tc.tile_pool(name="pm_pool", bufs=3) as pm_pool,
                tc.tile_pool(name="small", bufs=3) as small,
                tc.tile_pool(name="rec", bufs=2) as rec,
                tc.tile_pool(name="outstage", bufs=4) as outstage,
                tc.tile_pool(name="dscratch", bufs=4, space="DRAM") as dscratch,
                tc.tile_pool(name="psum_sc", bufs=3, space="PSUM") as psum_sc,
                tc.tile_pool(name="psum_att", bufs=2, space="PSUM") as psatt,
            ):
                for qt in range(SQ // NQ2):
                    q0 = qt * NQ2
                    mask_sb = mask_pool.tile([128, NKV, NQ2], FP16,
                                             tag="mask_sb")
                    nc.sync.dma_start(
                        out=mask_sb[:],
                        in_=maskt_d.ap().rearrange("(t p) q -> p t q", p=128)
                        [:, :, q0:q0 + NQ2])
                    for hp in range(JH):
                        hA, hB = 2 * hp, 2 * hp + 1
                        att_ps = {}
                        pm_big = {}
                        for h in (hA, hB):
                            att_ps[h] = psatt.tile(
                                [DH + 1, NQ2], F32, tag="att_ps",
                                name=f"attps_{qt}_{h}")
                            pm_big[h] = pm_pool.tile(
                                [128, NKV, NQ2], FP16, tag="pm",
                                name=f"pm_{qt}_{h}")
                        # software pipeline: scores(b) then attV(b-1), so the
                        # PE never waits on the exp/mask of the current batch
                        prev = None
                        for b in range(NB + 1):
                            if b < NB:
                                ps_b = {}
                                for h, base in ((hA, 0), (hB, 64)):
                                    ps = psum_sc.tile(
                                        [128, KB * NQ2], F32, tag="ps_sc",
                                        name=f"ps_{qt}_{hp}_{b}_{h}")
                                    for i in range(KB):
                                        kvt = b * KB + i
                                        nc.tensor.matmul(
                                            ps[:, i * NQ2:(i + 1) * NQ2],
                                            kt_sb[base:base + DH, hp,
                                                  kvt * 128:(kvt + 1) * 128],
                                            qt_sb[base:base + DH, hp,
                                                  q0:q0 + NQ2],
                                            start=True, stop=True)
                                    ps_b[h] = ps
                            if prev is not None:
                                bb, pm_prev = prev
                                for i in range(KB):
                                    kvt = bb * KB + i
                                    for h in (hA, hB):
                                        nc.tensor.matmul(
                                            att_ps[h][:],
                                            v_sb[:, kvt,
                                                 (h % HL) * (DH + 1):
                                                 (h % HL) * (DH + 1) + DH + 1],
                                            pm_big[h][:, kvt, :],
                                            start=(kvt == 0),
                                            stop=(kvt == NKV - 1))
                            if b < NB:
                                for h in (hA, hB):
                                    p_t = small.tile([128, KB * NQ2], FP16,
                                                     tag="p_exp")
                                    nc.scalar.activation(
                                        p_t[:], ps_b[h][:],
                                        mybir.ActivationFunctionType.Exp,
                                        scale=float(1.0 / np.sqrt(DH)))
                                    nc.vector.tensor_mul(
                                        pm_big[h][:, b * KB:(b + 1) * KB, :],
                                        p_t[:].rearrange(
                                            "p (t q) -> p t q", q=NQ2),
                                        mask_sb[:, b * KB:(b + 1) * KB, :])
                                prev = (b, None)
                        # recip of softmax sums; broadcast via DRAM roundtrip
                        for h in (hA, hB):
                            rrow = rec.tile([1, NQ2], F32, tag="rrow")
                            rscr = rec.tile([1, NQ2], F32, tag="rscr")
                            rre = rec.tile([1, NQ2], F32, tag="rrec")
                            nc.vector.tensor_copy(rrow[:],
                                                  att_ps[h][DH:DH + 1, :])
                            nc.vector.reciprocal_approx_accurate(
                                out=rre[:], in_=rrow[:], scratch=rscr[:])
                            rsc = dscratch.tile([1, NQ2], F32, tag="rsc",
                                                name=f"rsc_{qt}_{h}")
                            nc.sync.dma_start(out=rsc[:], in_=rre[:])
                            rb16 = rec.tile([128, NQ2], FP16, tag="rb16")
                            nc.gpsimd.dma_start(
                                out=rb16[:],
                                in_=rsc[:][0:1, :].partition_broadcast(128))
                            # normalize in place (one big DVE op), one DMA out
                            nc.vector.tensor_mul(
                                pm_big[h][:],
                                pm_big[h][:],
                                rb16[:, None, :].broadcast_to(
                                    [128, NKV, NQ2]))
                            nc.sync.dma_start(
                                out=awt_d.ap()[h].rearrange(
                                    "(t p) q -> p t q", p=128)
                                [:, :, q0:q0 + NQ2],
                                in_=pm_big[h][:])
                            nc.vector.tensor_mul(
                                attn_sb[(h % 2) * 64:(h % 2) * 64 + DH,
                                        h // 2, q0:q0 + NQ2],
                                att_ps[h][0:DH, :], rb16[0:DH, :])

                    # ---- output projection for this q quarter --------
                    for st in range(NQ2 // 128):
                        sq0 = q0 + st * 128
                        pso_w = psum_sc.tile([128, KB * NQ2], F32,
                                             tag="ps_sc",
                                             name=f"pso_{qt}_{st}")
                        pso = pso_w[:, 0:DIM_OUT]
                        for j in range(JH):
                            nc.tensor.matmul(
                                pso[:],
                                attn_sb[:, j, sq0:sq0 + 128],
                                wot_sb[:, j, :],
                                start=(j == 0), stop=(j == JH - 1))
                        oout = outstage.tile([128, DIM_OUT], F32, tag="oout")
                        nc.vector.tensor_add(oout[:], pso[:], bo_sb[:])
                        nc.sync.dma_start(out=outp_d.ap()[sq0:sq0 + 128, :],
                                          in_=oout[:])

    nc.compile()
    return nc


def _get_nc():
    global _CACHED_NC
    if _CACHED_NC is None:
        _CACHED_NC = _build_program()
    return _CACHED_NC


def _prep_core_inputs(c, Q, K, V, att_mask, Wq, bq, Wk, Wv, Wo, bo, bv):
    b, hh = c // 2, c % 2
    sl = slice(hh * S, (hh + 1) * S)
    keep = (~att_mask[b]).astype(np.float16)          # [SKV, SQ] 1=keep
    bo_eff = (bo / 2.0 + Wo[:, sl].astype(np.float64) @
              bv[sl].astype(np.float64)).astype(np.float32)
    return {
        "qt": np.ascontiguousarray(Q[b].T).astype(np.float16),
        "kt": np.ascontiguousarray(K[b].T).astype(np.float16),
        "vt": np.ascontiguousarray(V[b].T).astype(np.float16),
        "maskt": keep,
        "wqt": np.ascontiguousarray(Wq[sl, :].T).astype(np.float16),
        "wkt": np.ascontiguousarray(Wk[sl, :].T).astype(np.float16),
        "wvt": np.ascontiguousarray(Wv[sl, :].T).astype(np.float16),
        "wot": np.ascontiguousarray(Wo[:, sl].T).astype(np.float16),
        "bq": np.ascontiguousarray(
            bq[sl].astype(np.float32).reshape(S // 128, 128).T),
        "bo": bo_eff.reshape(1, DIM_OUT),
    }


def kernel(Q, K, V, att_mask, Wq, bq, Wk, bk, Wv, bv, Wo, bo, _trace=False):
    Q = np.asarray(Q, np.float32)
    K = np.asarray(K, np.float32)
    V = np.asarray(V, np.float32)
    att_mask = np.asarray(att_mask, bool)
    Wq, Wk, Wv, Wo = (np.asarray(a, np.float32) for a in (Wq, Wk, Wv, Wo))
    bq, bk, bv, bo = (np.asarray(a, np.float32) for a in (bq, bk, bv, bo))
    # bk drops out of softmax exactly; bv is folded into bo_eff.

    nc = _get_nc()
    in_maps = [_prep_core_inputs(c, Q, K, V, att_mask, Wq, bq, Wk, Wv, Wo,
                                 bo, bv) for c in range(N_CORES)]
    res = run_bass_kernel_spmd(nc, in_maps, core_ids=list(range(N_CORES)),
                               trace=_trace)

    att_weights = np.empty((B, H, SQ, SKV), np.float32)
    att_output = np.zeros((B, SQ, DIM_OUT), np.float32)
    try:
        import jax.numpy as jnp

        def tr(a):  # [HL, SKV, SQ] fp16 -> [HL, SQ, SKV] f32 (multithreaded)
            return np.asarray(jnp.transpose(jnp.asarray(a), (0, 2, 1))
                              .astype(jnp.float32))
    except Exception:
        def tr(a):
            return a.transpose(0, 2, 1).astype(np.float32)

    for c in range(N_CORES):
        b, hh = c // 2, c % 2
        att_weights[b, hh * HL:(hh + 1) * HL] = tr(res.results[c]["awt"])
        att_output[b] += res.results[c]["outp"]

    if _trace:
        kernel._last_results = res
    return att_output, att_weights
